# revision 24
# baseline (speedup 1.0000x reference)
"""Trainium2 Bass kernel for nn_Attention_4037269258732 (GQA attention with
RoPE, causal mask, and per-head sink-logit LSE renormalization).

Problem:  B=1, S=2048, DIM=2048, H=32 q-heads, KVH=8 kv-heads, HD=64.
          out = Wo @ attn(RoPE(Wq x), RoPE(Wk x), Wv x) + bo, causal,
          with out rows scaled by r = sumexp/(sumexp + e^sink).

Sharding (8 cores, tensor-parallel over heads):
  core c owns q-heads [4c, 4c+4), kv-head c, the matching rows of
  wq/wk/wv, wo's input-dim slice [256c, 256c+256), and sinks[4c:4c+4].
  Each core computes a full-shape [S, DIM] bf16 partial of the output
  projection (no bias); the host sums the 8 partials and adds wo_b.

v2 layout: merged software pipeline of 5 rounds.  Round r interleaves, in
PE program order, the QKV projection of sequence block r with the
attention of block r-1, so the PE never idles long enough to trip the
HAM clock gate and the ACT-engine exp evictions overlap matmuls.

  - scores: per head-pair, two row-packed K=64 matmuls into one
    [128,1024] fp32 PSUM tile (2 banks); a single wide ACT Exp evicts
    both heads at once (amortizes the 352-cycle ACT fixed cost).
  - PV: col-tiled M=64 pairs (tile_position (0,0)/(0,64)) accumulate two
    heads into one PSUM bank concurrently; per-head softmax denominators
    via 4 concurrent M=1 matmuls (cols 0/32/64/96) into one bank.
  - PSUM budget (8 banks): S-ring 2x[128,1024] (4) shared by scores,
    V-transposes, renorm broadcasts and outproj accumulators; proj 1;
    pso 2; denom 1.
"""

import numpy as np
import ml_dtypes

import bass_rust
import concourse.bass as bass
import concourse.tile as tile
from concourse import mybir
from concourse.bass_utils import run_bass_kernel_spmd

F32 = mybir.dt.float32
BF16 = mybir.dt.bfloat16
AF = mybir.ActivationFunctionType
OP = mybir.AluOpType
BF = ml_dtypes.bfloat16

B, S, DIM = 1, 2048, 2048
H, KVH, HD = 32, 8, 64
NCORES = 8
QH = H // NCORES          # 4 q heads per core
SBLK = 512                # sq block size
NSB = S // SBLK           # 4
NDC = DIM // 128          # 16 contraction chunks
NST = S // 128            # 16 sk tiles
SCALE = 1.0 / 8.0         # 1/sqrt(HD)

_ws_ctr = [0]


def _fix_range_clears(nc):
    """walrus here rejects the EVENT_SEMAPHORE_RANGE_CLEAR ISA struct
    ("ISA wrong length"); replace with per-sem write-0 NoOps."""
    import re as _re
    for f in nc.m.functions:
        for blk in f.blocks:
            out, changed = [], False
            for inst in blk.instructions:
                if type(inst).__name__ == "InstISA" and inst.isa_opcode == 176:
                    m = _re.search(r"range_first=(\d+) range_last=(\d+)", inst.concise())
                    first, last = int(m.group(1)), int(m.group(2))
                    for semid in range(first, last + 1):
                        _ws_ctr[0] += 1
                        nop = mybir.InstNoOp(name=f"I-rc-{_ws_ctr[0]}", ins=[], outs=[])
                        nop.engine = inst.engine
                        nop.sync_info = bass_rust.SyncInfo(
                            on_wait=[],
                            on_update=[
                                bass_rust.SyncUpdate(
                                    sync_type="semaphore",
                                    id=semid,
                                    update_mode="sem-wr-imm",
                                    update_value=0,
                                )
                            ],
                        )
                        out.append(nop)
                    changed = True
                    continue
                out.append(inst)
            if changed:
                blk.instructions = out


def _split_excess_waits(nc, max_waits=1):
    """walrus on this image encodes at most one SyncWait per instruction;
    hoist excess waits onto same-engine NoOps placed just before."""
    for f in nc.m.functions:
        for blk in f.blocks:
            out, changed = [], False
            for inst in blk.instructions:
                si = inst.sync_info
                waits = list(si.on_wait) if si is not None else []
                if len(waits) > max_waits:
                    excess, keep = waits[:-max_waits], waits[-max_waits:]
                    for k in range(0, len(excess), max_waits):
                        _ws_ctr[0] += 1
                        nop = mybir.InstNoOp(name=f"I-ws-{_ws_ctr[0]}", ins=[], outs=[])
                        nop.engine = inst.engine
                        nop.sync_info = bass_rust.SyncInfo(
                            on_wait=excess[k : k + max_waits], on_update=[]
                        )
                        out.append(nop)
                    inst.sync_info = bass_rust.SyncInfo(
                        on_wait=keep, on_update=list(si.on_update)
                    )
                    changed = True
                out.append(inst)
            if changed:
                blk.instructions = out


def prep_inputs(inputs):
    """Host-side sharding/layout prep. Returns per-core input maps."""
    x = np.asarray(inputs["x"], np.float32)
    rope = np.asarray(inputs["rope_cache"], np.float32)
    wq = np.asarray(inputs["wq_w"], np.float32)
    bq = np.asarray(inputs["wq_b"], np.float32)
    wk = np.asarray(inputs["wk_w"], np.float32)
    bk = np.asarray(inputs["wk_b"], np.float32)
    wv = np.asarray(inputs["wv_w"], np.float32)
    bv = np.asarray(inputs["wv_b"], np.float32)
    wo = np.asarray(inputs["wo_w"], np.float32)
    sinks = np.asarray(inputs["sinks"], np.float32)

    xT = np.ascontiguousarray(x[0].T).astype(BF)            # [DIM, S]
    cosT = rope[:, :HD].T                                   # [64, S]
    sinT = rope[:, HD:].T
    cos2 = np.ascontiguousarray(np.concatenate([cosT, cosT], 0)).astype(BF)
    # sin_rot indexed by SOURCE partition: source rows hd in [0,32) land at
    # out rows hd+32 with +sin[hd+32]; source rows hd in [32,64) land at
    # out rows hd-32 with -sin[hd-32]. Duplicated for both heads per tile.
    sr = np.concatenate([sinT[32:64], -sinT[0:32]], 0)      # [64, S]
    sin_rot2 = np.ascontiguousarray(np.concatenate([sr, sr], 0)).astype(BF)
    tri = np.triu(np.ones((128, 128), BF))                  # mask[p, j] = j >= p
    ident = np.eye(HD, dtype=np.float32)
    ones_col = np.ones((128, 64), BF)
    ones_f = np.ones((128, 64), np.float32)

    in_maps = []
    for c in range(NCORES):
        qs = slice(c * QH * HD, (c + 1) * QH * HD)          # 256 q rows
        ks = slice(c * HD, (c + 1) * HD)                    # 64 kv rows
        # wproj columns: [q 256 | k 64 | v 64] = 384
        wproj = np.concatenate([wq[qs].T, wk[ks].T, wv[ks].T], axis=1)
        bcol = np.zeros((128, 3), np.float32)
        bcol[:, 0] = bq[qs][0:128]
        bcol[:, 1] = bq[qs][128:256]
        bcol[0:64, 2] = bk[ks]
        bcol[64:128, 2] = bv[ks]
        woT = np.ascontiguousarray(wo[:, qs].T).astype(BF)  # [256, DIM]
        esc = np.zeros((128, 1), np.float32)
        for h in range(QH):
            esc[32 * h, 0] = np.exp(sinks[c * QH + h])
        in_maps.append(
            {
                "xT": xT,
                "wproj": np.ascontiguousarray(wproj).astype(BF),
                "bproj": bcol,
                "cos2": cos2,
                "sinr2": sin_rot2,
                "woT": woT,
                "esinkc": esc,
                "tri": tri,
                "identf": ident,
                "onesb": ones_col,
                "onesf": ones_f,
            }
        )
    return in_maps


def build_nc(split_waits=True):
    nc = bass.Bass("TRN2", target_bir_lowering=False, debug=False, num_devices=NCORES)
    xT = nc.dram_tensor("xT", [DIM, S], BF16, kind="ExternalInput").ap()
    wproj = nc.dram_tensor("wproj", [DIM, 384], BF16, kind="ExternalInput").ap()
    bproj = nc.dram_tensor("bproj", [128, 3], F32, kind="ExternalInput").ap()
    cos2 = nc.dram_tensor("cos2", [128, S], BF16, kind="ExternalInput").ap()
    sinr2 = nc.dram_tensor("sinr2", [128, S], BF16, kind="ExternalInput").ap()
    woT = nc.dram_tensor("woT", [2 * 128, DIM], BF16, kind="ExternalInput").ap()
    esinkc = nc.dram_tensor("esinkc", [128, 1], F32, kind="ExternalInput").ap()
    tri = nc.dram_tensor("tri", [128, 128], BF16, kind="ExternalInput").ap()
    identf = nc.dram_tensor("identf", [HD, HD], F32, kind="ExternalInput").ap()
    onesb = nc.dram_tensor("onesb", [128, 64], BF16, kind="ExternalInput").ap()
    onesf = nc.dram_tensor("onesf", [128, 64], F32, kind="ExternalInput").ap()
    out = nc.dram_tensor("out", [S, DIM], BF16, kind="ExternalOutput").ap()

    with tile.TileContext(nc) as tc:
        with (
            tc.tile_pool(name="persist", bufs=1) as P,
            tc.tile_pool(name="ps_s", bufs=2, space="PSUM") as PS_S,
            tc.tile_pool(name="tmp", bufs=2) as TMP,
            tc.tile_pool(name="ptp", bufs=10) as PT,
            tc.tile_pool(name="evp", bufs=4) as EV,
            tc.tile_pool(name="rnp", bufs=2) as RN,
        ):
            PS_P = tc.alloc_tile_pool(name="ps_p", bufs=1, space="PSUM")
            PS_O = tc.alloc_tile_pool(name="ps_o", bufs=2, space="PSUM")
            PS_D = tc.alloc_tile_pool(name="ps_d", bufs=1, space="PSUM")
            # ---- persistent tiles ----
            esink_t = P.tile([128, 1], F32, tag="esink")
            tri_t = P.tile([128, 128], BF16, tag="tri")
            wo_t = [P.tile([128, DIM], BF16, name=f"wo{i}", tag=f"wo{i}") for i in range(2)]
            qp = [P.tile([128, S], BF16, name=f"qp{i}", tag=f"qp{i}") for i in range(2)]
            kT2 = P.tile([128, S], BF16, tag="kT2")
            vTf = P.tile([64, S], F32, tag="vTf")
            vext = P.tile([128, NST * 64], BF16, tag="vext")
            outstk = [P.tile([128, S], BF16, name=f"os{i}", tag=f"os{i}") for i in range(2)]
            id_t = P.tile([HD, HD], F32, tag="idp")
            onesb_t = P.tile([128, 64], BF16, tag="onesb_t")
            onesf_t = P.tile([128, 64], F32, tag="onesf_t")
            bcol_t = P.tile([128, 3], F32, tag="bcol")
            cos_t = P.tile([128, S], BF16, tag="cos")
            sinr_t = P.tile([128, S], BF16, tag="sinr")
            scr = P.tile([1, 16], F32, tag="scr")
            w_t = [P.tile([128, 384], BF16, name=f"w{dc}", tag=f"w{dc}") for dc in range(NDC)]
            x_t = [P.tile([128, S], BF16, name=f"x{dc}", tag=f"x{dc}") for dc in range(NDC)]
            # weights + first x column block interleaved on the two HWDGE
            # queues so round-0 projection unblocks per-dc as data lands;
            # later column blocks follow in round order
            _xq = [nc.sync, nc.scalar]
            for dc in range(NDC):
                q = _xq[dc % 2]
                q.dma_start(w_t[dc][:], wproj[dc * 128 : (dc + 1) * 128, :])
                q.dma_start(x_t[dc][:, 0:SBLK], xT[dc * 128 : (dc + 1) * 128, 0:SBLK])
            # block 1 rides both fast queues right after block 0; blocks 2-3
            # queue strictly behind on sync so they cannot steal HBM bandwidth
            # from the critical early loads
            cs = slice(SBLK, 2 * SBLK)
            for dc in range(NDC):
                _xq[dc % 2].dma_start(x_t[dc][:, cs], xT[dc * 128 : (dc + 1) * 128, cs])
            for blk in range(2, NSB):
                cs = slice(blk * SBLK, (blk + 1) * SBLK)
                for dc in range(NDC):
                    nc.sync.dma_start(x_t[dc][:, cs], xT[dc * 128 : (dc + 1) * 128, cs])
            nc.gpsimd.dma_start(bcol_t[:], bproj[:])
            nc.gpsimd.dma_start(cos_t[:], cos2[:])
            nc.gpsimd.dma_start(sinr_t[:], sinr2[:])
            nc.gpsimd.dma_start(id_t[:], identf[:])
            nc.gpsimd.dma_start(onesb_t[:], onesb[:])
            nc.gpsimd.dma_start(onesf_t[:], onesf[:])
            nc.gpsimd.dma_start(esink_t[:], esinkc[:])
            nc.gpsimd.dma_start(tri_t[:], tri[:])
            for i in range(2):
                nc.gpsimd.dma_start(wo_t[i][:], woT[i * 128 : (i + 1) * 128, :])
            # pull the ACT Exp/Ln table load off the critical path
            nc.scalar.activation(scr[0:1, 0:3], bcol_t[0:1, 0:3], AF.Exp)
            nc.scalar.activation(scr[0:1, 0:3], scr[0:1, 0:3], AF.Ln)
            nc.scalar.activation(scr[0:1, 0:3], scr[0:1, 0:3], AF.Identity)

            # ---------------- helpers (emit instructions) ----------------

            def rope_q(i, pp, ss):
                """Evict q tile i from psum pp (one fast read), then RoPE in
                bf16 on SBUF (2x DVE mode); frees the proj psum bank early."""
                u = TMP.tile([128, SBLK], BF16, name="u", tag="u")
                nc.scalar.activation(u[:], pp, AF.Identity, bias=bcol_t[:, i : i + 1])
                t1 = TMP.tile([128, SBLK], BF16, name="t1", tag="t1")
                nc.vector.tensor_tensor(t1[:], u[:], cos_t[:, ss], op=OP.mult)
                t2 = TMP.tile([128, SBLK], BF16, name="t2", tag="t2")
                for g in range(4):
                    d0 = 32 * g
                    s0 = d0 + 32 if g % 2 == 0 else d0 - 32
                    nc.vector.tensor_tensor(
                        t2[d0 : d0 + 32, :], u[s0 : s0 + 32, :],
                        sinr_t[s0 : s0 + 32, ss], op=OP.mult,
                    )
                nc.vector.tensor_tensor(qp[i][:, ss], t1[:], t2[:], op=OP.add)

            def rope_kv(pp, ss):
                uk = TMP.tile([64, SBLK], BF16, name="uk", tag="uk")
                nc.scalar.activation(uk[:], pp[0:64, :], AF.Identity, bias=bcol_t[0:64, 2:3])
                # v rows with bias, fp32 (transposed later on PE)
                nc.scalar.activation(vTf[:, ss], pp[64:128, :], AF.Identity, bias=bcol_t[64:128, 2:3])
                tk1 = TMP.tile([64, SBLK], BF16, name="tk1", tag="tk1")
                nc.vector.tensor_tensor(tk1[:], uk[:], cos_t[0:64, ss], op=OP.mult)
                tk2 = TMP.tile([64, SBLK], BF16, name="tk2", tag="tk2")
                nc.vector.tensor_tensor(
                    tk2[0:32, :], uk[32:64, :], sinr_t[32:64, ss], op=OP.mult
                )
                nc.vector.tensor_tensor(
                    tk2[32:64, :], uk[0:32, :], sinr_t[0:32, ss], op=OP.mult
                )
                nc.vector.tensor_tensor(kT2[0:64, ss], tk1[:], tk2[:], op=OP.add)
                nc.vector.tensor_copy(kT2[64:128, ss], kT2[0:64, ss])

            def transp_round(r):
                """Transpose this round's 4 v tiles into vext via the S ring."""
                tp = PS_S.tile([128, 1024], F32, name="tp", tag="s")
                for j in range(4):
                    t = 4 * r + j
                    nc.tensor.transpose(
                        tp[:, j * 64 : (j + 1) * 64],
                        vTf[:, t * 128 : (t + 1) * 128],
                        id_t[:],
                    )
                    nc.vector.tensor_copy(
                        vext[:, t * 64 : (t + 1) * 64], tp[:, j * 64 : (j + 1) * 64]
                    )

            pso_cur = [None]
            den_cur = [None]
            ptt_store = {}

            def s_iter(b, t):
                """Scores + exp for sk-tile t of block b (both head pairs)."""
                off = 128 * (t - 4 * b) if t >= 4 * b else 0
                n0 = b * SBLK + off
                ptts = []
                for pi in range(2):
                    sbt = PS_S.tile([128, 1024], F32, name="sbt", tag="s")
                    # lane0 at [off:512], lane1 packed at [512:1024-off] so the
                    # exp input region is contiguous (no stale-data gap)
                    for lane, (c0, c1) in enumerate([(off, 512), (512, 1024 - off)]):
                        nc.tensor.matmul(
                            sbt[:, c0:c1],
                            kT2[64 * lane : 64 * lane + 64, t * 128 : (t + 1) * 128],
                            qp[pi][64 * lane : 64 * lane + 64, n0 : (b + 1) * SBLK],
                            start=True, stop=True,
                            tile_position=(64 * lane, 0),
                        )
                    ptt = PT.tile([128, 1024], BF16, name="ptt", tag="pt")
                    nc.scalar.activation(
                        ptt[:, off : 1024 - off], sbt[:, off : 1024 - off],
                        AF.Exp, scale=SCALE,
                    )
                    if t >= 4 * b:
                        for c in (off, 512):
                            nc.vector.tensor_tensor(
                                ptt[:, c : c + 128], ptt[:, c : c + 128],
                                tri_t[:], op=OP.mult,
                            )
                    ptts.append(ptt)
                ptt_store[(b, t)] = ptts

            def pv_iter(b, t):
                """PV accumulation + denominators for sk-tile t of block b."""
                off = 128 * (t - 4 * b) if t >= 4 * b else 0
                ptts = ptt_store.pop((b, t))
                if t == 0:
                    pso_cur[0] = [
                        PS_O.tile([128, SBLK], F32, name=f"pso{pi}", tag="o")
                        for pi in range(2)
                    ]
                    den_cur[0] = PS_D.tile([128, SBLK], F32, name="den", tag="d")
                pso, den = pso_cur[0], den_cur[0]
                vx = vext[:, t * 64 : (t + 1) * 64]
                first = t == 0
                last = t == 4 * b + 3
                for pi in range(2):
                    nc.tensor.matmul(
                        pso[pi][0:64, off:SBLK],
                        vx, ptts[pi][:, off:512],
                        start=first, stop=last, tile_position=(0, 0),
                    )
                    # skip_group_check: the sim's zero-region bookkeeping
                    # mis-translates partition-offset APs (aliases partitions
                    # 8..71); flags here mirror the checked even-half matmul
                    nc.tensor.matmul(
                        pso[pi][64:128, off:SBLK],
                        vx, ptts[pi][:, 512 : 1024 - off],
                        start=first, stop=last, tile_position=(0, 64),
                        skip_group_check=True,
                    )
                for h in range(QH):
                    pi, odd = h // 2, h % 2
                    # M=32 (same cycles as M=1) so the whole den bank is
                    # initialized for the full-tile renorm read later
                    nc.tensor.matmul(
                        den[32 * h : 32 * h + 32, off:SBLK],
                        onesb_t[:, 0:32],
                        ptts[pi][:, 512 * odd + off * (1 - odd) : 512 + 512 * odd - off * odd],
                        start=first, stop=last, tile_position=(0, 32 * h),
                        skip_group_check=(h > 0),
                    )

            rinv_cur = [None]

            def renorm_a(b):
                """r = sumexp + e^sink; rinv = 1/r (ACT chain, no PE)."""
                den = den_cur[0]
                rowb = RN.tile([128, SBLK], F32, name="rowb", tag="rowb")
                nc.vector.tensor_scalar_add(rowb[:], den[:], esink_t[:])
                lnr = RN.tile([128, SBLK], F32, name="lnr", tag="lnr")
                nc.scalar.activation(lnr[:], rowb[:], AF.Ln)
                rinv = RN.tile([128, SBLK], F32, name="rinv", tag="rinv")
                nc.scalar.activation(rinv[:], lnr[:], AF.Exp, scale=-1.0)
                rinv_cur[0] = rinv

            def renorm_b(b):
                """outstk = pso * rinv broadcast (K=1 matmuls + DVE mults)."""
                bs = slice(b * SBLK, (b + 1) * SBLK)
                pso, rinv = pso_cur[0], rinv_cur[0]
                rbt = PS_S.tile([128, 1024], F32, name="rbt", tag="s")
                for h in range(QH):
                    pi, odd = h // 2, h % 2
                    nc.tensor.matmul(
                        rbt[64 * odd : 64 * odd + 64, 512 * pi : 512 * pi + 512],
                        onesf_t[32 * h : 32 * h + 1, :], rinv[32 * h : 32 * h + 1, :],
                        start=True, stop=True, tile_position=(32 * h, 64 * odd),
                    )
                rbs = RN.tile([128, 1024], F32, name="rbs", tag="rbs")
                nc.vector.tensor_copy(rbs[:, 0:512], rbt[:, 0:512])
                nc.scalar.copy(rbs[:, 512:1024], rbt[:, 512:1024])
                for pi in range(2):
                    nc.vector.tensor_tensor(
                        outstk[pi][:, bs], pso[pi][:],
                        rbs[:, 512 * pi : 512 * pi + 512], op=OP.mult,
                    )

            def renorm_tail(b):
                renorm_a(b)
                renorm_b(b)

            def outproj_tile(st, dpair, split_cast=False, pool=None):
                """Project sq tile st for output column pair dpair (2x512)."""
                psf = None
                if pool is None:
                    psf = PS_S.tile([128, 1024], F32, name="psf", tag="s")
                for half in range(2):
                    db = 2 * dpair + half
                    ds = slice(db * SBLK, (db + 1) * SBLK)
                    if pool is not None:
                        psf_h = pool.tile([128, SBLK], F32, name="psfh", tag="f")
                        hs = psf_h[:]
                    else:
                        hs = psf[:, 512 * half : 512 * half + 512]
                    nc.tensor.matmul(
                        hs,
                        outstk[0][:, st * 128 : (st + 1) * 128], wo_t[0][:, ds],
                        start=True, stop=False,
                    )
                    nc.tensor.matmul(
                        hs,
                        outstk[1][:, st * 128 : (st + 1) * 128], wo_t[1][:, ds],
                        start=False, stop=True,
                    )
                    ot = EV.tile([128, SBLK], BF16, name="ot", tag="ev")
                    if split_cast and half == 1:
                        nc.scalar.copy(ot[:], hs)
                    else:
                        nc.vector.tensor_copy(ot[:], hs)
                    _oq = nc.sync if half == 0 else nc.gpsimd
                    _oq.dma_start(out[st * 128 : (st + 1) * 128, ds], ot[:])

            def proj_group(g, ss, pp):
                c0 = 128 * g if g < 2 else 256
                c1 = c0 + 128
                for dc in range(NDC):
                    nc.tensor.matmul(
                        pp, w_t[dc][:, c0:c1], x_t[dc][:, ss],
                        start=(dc == 0), stop=(dc == NDC - 1),
                    )

            # ---------------- round 0: projection of block 0 ----------------
            ss0 = slice(0, SBLK)
            s_q = PS_S.tile([128, 1024], F32, name="s_q", tag="s")
            s_k = PS_S.tile([128, 1024], F32, name="s_k", tag="s")
            for dc in range(NDC):
                nc.tensor.matmul(
                    s_q[:, 0:512], w_t[dc][:, 0:128], x_t[dc][:, ss0],
                    start=(dc == 0), stop=(dc == NDC - 1),
                )
                nc.tensor.matmul(
                    s_q[:, 512:1024], w_t[dc][:, 128:256], x_t[dc][:, ss0],
                    start=(dc == 0), stop=(dc == NDC - 1),
                )
                nc.tensor.matmul(
                    s_k[:, 0:512], w_t[dc][:, 256:384], x_t[dc][:, ss0],
                    start=(dc == 0), stop=(dc == NDC - 1),
                )
            rope_q(0, s_q[:, 0:512], ss0)
            rope_q(1, s_q[:, 512:1024], ss0)
            rope_kv(s_k[:, 0:512], ss0)
            transp_round(0)
            s_iter(0, 0)
            s_iter(0, 1)

            # ------- rounds 1..3: proj r + attn r-1, score-lead pipeline -----
            for r in range(1, NSB):
                b = r - 1
                nt = 4 * b + 4
                ssr = slice(r * SBLK, (r + 1) * SBLK)

                fillers = []

                def mk_proj(g, ssr=ssr):
                    def f():
                        pp = PS_P.tile([128, SBLK], F32, name="pp", tag="p")
                        proj_group(g, ssr, pp[:])
                        if g < 2:
                            rope_q(g, pp[:], ssr)
                        else:
                            rope_kv(pp[:], ssr)
                    return f

                for g in range(3):
                    fillers.append(mk_proj(g))
                if b >= 1:
                    for st in range(4 * (b - 1), 4 * (b - 1) + 4):
                        for dp in range(2):
                            fillers.append(
                                lambda st=st, dp=dp: outproj_tile(st, dp)
                            )
                fi = 0
                for i in range(nt):
                    if i + 2 < nt:
                        s_iter(b, i + 2)
                    pv_iter(b, i)
                    while fi < len(fillers) and (fi + 1) * nt <= (i + 1) * len(fillers):
                        fillers[fi]()
                        fi += 1
                while fi < len(fillers):
                    fillers[fi]()
                    fi += 1
                renorm_a(b)
                transp_round(r)
                s_iter(r, 0)
                s_iter(r, 1)
                renorm_b(b)

            # ------- block 3: lead-4 score prefetch so the last PV iters
            # run dense (exps pre-done), outproj(2) tiles as spacers -------
            ofill = [
                (lambda st=st, dp=dp: outproj_tile(st, dp))
                for st in range(8, 12) for dp in range(2)
            ]
            ofill[0]()
            s_iter(3, 2)
            ofill[1]()
            s_iter(3, 3)
            for i in range(16):
                pv_iter(3, i)
                if i + 4 < 16:
                    s_iter(3, i + 4)
                if i in (1, 3, 5, 7, 9, 11):
                    ofill[2 + i // 2]()

            # ---------------- final renorm + output projection ----------------
            # scratch matmuls keep the PE (and HAM clock) busy while the
            # serial renorm chain runs on ACT/DVE; output is never read
            scratch = PS_S.tile([128, 1024], F32, name="scratch", tag="s")
            for w in range(16):
                nc.tensor.matmul(
                    scratch[0:64, 0:512], onesb_t[:, 0:64], qp[0][0:128, 0:512],
                    start=True, stop=True,
                )
            renorm_tail(3)
            PS_D.release()
            PS_O.release()
            PS_P.release()
            PS_F = tc.alloc_tile_pool(name="ps_f", bufs=4, space="PSUM")
            for st in range(12, 16):
                for dp in range(2):
                    outproj_tile(st, dp, split_cast=True, pool=PS_F)
            PS_F.release()

    _fix_range_clears(nc)
    if split_waits:
        _split_excess_waits(nc)
    return nc


_nc_cache = [None]


def kernel(**inputs):
    in_maps = prep_inputs(inputs)
    if _nc_cache[0] is None:
        _nc_cache[0] = build_nc()
    nc = _nc_cache[0]
    res = run_bass_kernel_spmd(nc, in_maps, list(range(NCORES)))
    acc = res.results[0]["out"].astype(np.float32)
    for i in range(1, NCORES):
        acc = acc + res.results[i]["out"].astype(np.float32)
    acc = acc + np.asarray(inputs["wo_b"], np.float32).reshape(1, DIM)
    return acc.reshape(B, S, DIM)


# revision 26
# speedup vs baseline: 1.1523x; 1.1523x over previous
"""Trainium2 Bass kernel for nn_Attention_4037269258732 (GQA attention with
RoPE, causal mask, and per-head sink-logit LSE renormalization).

Problem:  B=1, S=2048, DIM=2048, H=32 q-heads, KVH=8 kv-heads, HD=64.
          out = Wo @ attn(RoPE(Wq x), RoPE(Wk x), Wv x) + bo, causal,
          with out rows scaled by r = sumexp/(sumexp + e^sink).

Sharding (8 cores, tensor-parallel over heads):
  core c owns q-heads [4c, 4c+4), kv-head c, the matching rows of
  wq/wk/wv, wo's input-dim slice [256c, 256c+256), and sinks[4c:4c+4].
  Each core computes a full-shape [S, DIM] bf16 partial of the output
  projection (no bias); the host sums the 8 partials and adds wo_b.

v2 layout: merged software pipeline of 5 rounds.  Round r interleaves, in
PE program order, the QKV projection of sequence block r with the
attention of block r-1, so the PE never idles long enough to trip the
HAM clock gate and the ACT-engine exp evictions overlap matmuls.

  - scores: per head-pair, two row-packed K=64 matmuls into one
    [128,1024] fp32 PSUM tile (2 banks); a single wide ACT Exp evicts
    both heads at once (amortizes the 352-cycle ACT fixed cost).
  - PV: col-tiled M=64 pairs (tile_position (0,0)/(0,64)) accumulate two
    heads into one PSUM bank concurrently; per-head softmax denominators
    via 4 concurrent M=1 matmuls (cols 0/32/64/96) into one bank.
  - PSUM budget (8 banks): S-ring 2x[128,1024] (4) shared by scores,
    V-transposes, renorm broadcasts and outproj accumulators; proj 1;
    pso 2; denom 1.
"""

import numpy as np
import ml_dtypes

import bass_rust
import concourse.bass as bass
import concourse.tile as tile
from concourse import mybir
from concourse.bass_utils import run_bass_kernel_spmd

F32 = mybir.dt.float32
BF16 = mybir.dt.bfloat16
AF = mybir.ActivationFunctionType
OP = mybir.AluOpType
BF = ml_dtypes.bfloat16

B, S, DIM = 1, 2048, 2048
H, KVH, HD = 32, 8, 64
NCORES = 8
QH = H // NCORES          # 4 q heads per core
SBLK = 512                # sq block size
NSB = S // SBLK           # 4
NDC = DIM // 128          # 16 contraction chunks
NST = S // 128            # 16 sk tiles
SCALE = 1.0 / 8.0         # 1/sqrt(HD)

_ws_ctr = [0]


def _fix_range_clears(nc):
    """walrus here rejects the EVENT_SEMAPHORE_RANGE_CLEAR ISA struct
    ("ISA wrong length"); replace with per-sem write-0 NoOps."""
    import re as _re
    for f in nc.m.functions:
        for blk in f.blocks:
            out, changed = [], False
            for inst in blk.instructions:
                if type(inst).__name__ == "InstISA" and inst.isa_opcode == 176:
                    m = _re.search(r"range_first=(\d+) range_last=(\d+)", inst.concise())
                    first, last = int(m.group(1)), int(m.group(2))
                    for semid in range(first, last + 1):
                        _ws_ctr[0] += 1
                        nop = mybir.InstNoOp(name=f"I-rc-{_ws_ctr[0]}", ins=[], outs=[])
                        nop.engine = inst.engine
                        nop.sync_info = bass_rust.SyncInfo(
                            on_wait=[],
                            on_update=[
                                bass_rust.SyncUpdate(
                                    sync_type="semaphore",
                                    id=semid,
                                    update_mode="sem-wr-imm",
                                    update_value=0,
                                )
                            ],
                        )
                        out.append(nop)
                    changed = True
                    continue
                out.append(inst)
            if changed:
                blk.instructions = out


def _split_excess_waits(nc, max_waits=1):
    """walrus on this image encodes at most one SyncWait per instruction;
    hoist excess waits onto same-engine NoOps placed just before."""
    for f in nc.m.functions:
        for blk in f.blocks:
            out, changed = [], False
            for inst in blk.instructions:
                si = inst.sync_info
                waits = list(si.on_wait) if si is not None else []
                if len(waits) > max_waits:
                    excess, keep = waits[:-max_waits], waits[-max_waits:]
                    for k in range(0, len(excess), max_waits):
                        _ws_ctr[0] += 1
                        nop = mybir.InstNoOp(name=f"I-ws-{_ws_ctr[0]}", ins=[], outs=[])
                        nop.engine = inst.engine
                        nop.sync_info = bass_rust.SyncInfo(
                            on_wait=excess[k : k + max_waits], on_update=[]
                        )
                        out.append(nop)
                    inst.sync_info = bass_rust.SyncInfo(
                        on_wait=keep, on_update=list(si.on_update)
                    )
                    changed = True
                out.append(inst)
            if changed:
                blk.instructions = out


def prep_inputs(inputs):
    """Host-side sharding/layout prep. Returns per-core input maps."""
    x = np.asarray(inputs["x"], np.float32)
    rope = np.asarray(inputs["rope_cache"], np.float32)
    wq = np.asarray(inputs["wq_w"], np.float32)
    bq = np.asarray(inputs["wq_b"], np.float32)
    wk = np.asarray(inputs["wk_w"], np.float32)
    bk = np.asarray(inputs["wk_b"], np.float32)
    wv = np.asarray(inputs["wv_w"], np.float32)
    bv = np.asarray(inputs["wv_b"], np.float32)
    wo = np.asarray(inputs["wo_w"], np.float32)
    sinks = np.asarray(inputs["sinks"], np.float32)

    xT = np.ascontiguousarray(x[0].T).astype(BF)            # [DIM, S]
    cosT = rope[:, :HD].T                                   # [64, S]
    sinT = rope[:, HD:].T
    cos2 = np.ascontiguousarray(np.concatenate([cosT, cosT], 0)).astype(BF)
    # sin_rot indexed by SOURCE partition: source rows hd in [0,32) land at
    # out rows hd+32 with +sin[hd+32]; source rows hd in [32,64) land at
    # out rows hd-32 with -sin[hd-32]. Duplicated for both heads per tile.
    sr = np.concatenate([sinT[32:64], -sinT[0:32]], 0)      # [64, S]
    sin_rot2 = np.ascontiguousarray(np.concatenate([sr, sr], 0)).astype(BF)
    tri = np.triu(np.ones((128, 128), BF))                  # mask[p, j] = j >= p
    ident = np.eye(HD, dtype=np.float32)
    ones_col = np.ones((128, 64), BF)
    ones_f = np.ones((128, 64), np.float32)

    in_maps = []
    for c in range(NCORES):
        qs = slice(c * QH * HD, (c + 1) * QH * HD)          # 256 q rows
        ks = slice(c * HD, (c + 1) * HD)                    # 64 kv rows
        # wproj columns: [q 256 | k 64 | v 64] = 384
        wproj = np.concatenate([wq[qs].T, wk[ks].T, wv[ks].T], axis=1)
        bcol = np.zeros((128, 3), np.float32)
        bcol[:, 0] = bq[qs][0:128]
        bcol[:, 1] = bq[qs][128:256]
        bcol[0:64, 2] = bk[ks]
        bcol[64:128, 2] = bv[ks]
        woT = np.ascontiguousarray(wo[:, qs].T).astype(BF)  # [256, DIM]
        esc = np.zeros((128, 1), np.float32)
        for h in range(QH):
            esc[32 * h, 0] = np.exp(sinks[c * QH + h])
        in_maps.append(
            {
                "xT": xT,
                "wproj": np.ascontiguousarray(wproj).astype(BF),
                "bproj": bcol,
                "cos2": cos2,
                "sinr2": sin_rot2,
                "woT": woT,
                "esinkc": esc,
                "tri": tri,
                "identf": ident,
                "onesb": ones_col,
                "onesf": ones_f,
            }
        )
    return in_maps


def build_nc(split_waits=True):
    nc = bass.Bass("TRN2", target_bir_lowering=False, debug=False, num_devices=NCORES)
    xT = nc.dram_tensor("xT", [DIM, S], BF16, kind="ExternalInput").ap()
    wproj = nc.dram_tensor("wproj", [DIM, 384], BF16, kind="ExternalInput").ap()
    bproj = nc.dram_tensor("bproj", [128, 3], F32, kind="ExternalInput").ap()
    cos2 = nc.dram_tensor("cos2", [128, S], BF16, kind="ExternalInput").ap()
    sinr2 = nc.dram_tensor("sinr2", [128, S], BF16, kind="ExternalInput").ap()
    woT = nc.dram_tensor("woT", [2 * 128, DIM], BF16, kind="ExternalInput").ap()
    esinkc = nc.dram_tensor("esinkc", [128, 1], F32, kind="ExternalInput").ap()
    tri = nc.dram_tensor("tri", [128, 128], BF16, kind="ExternalInput").ap()
    identf = nc.dram_tensor("identf", [HD, HD], F32, kind="ExternalInput").ap()
    onesb = nc.dram_tensor("onesb", [128, 64], BF16, kind="ExternalInput").ap()
    onesf = nc.dram_tensor("onesf", [128, 64], F32, kind="ExternalInput").ap()
    out = nc.dram_tensor("out", [S, DIM], BF16, kind="ExternalOutput").ap()

    with tile.TileContext(nc) as tc:
        with (
            tc.tile_pool(name="persist", bufs=1) as P,
            tc.tile_pool(name="ps_s", bufs=2, space="PSUM") as PS_S,
            tc.tile_pool(name="tmp", bufs=2) as TMP,
            tc.tile_pool(name="ptp", bufs=10) as PT,
            tc.tile_pool(name="evp", bufs=4) as EV,
            tc.tile_pool(name="rnp", bufs=2) as RN,
        ):
            PS_P = tc.alloc_tile_pool(name="ps_p", bufs=1, space="PSUM")
            PS_O = tc.alloc_tile_pool(name="ps_o", bufs=2, space="PSUM")
            PS_D = tc.alloc_tile_pool(name="ps_d", bufs=1, space="PSUM")
            # ---- persistent tiles ----
            esink_t = P.tile([128, 1], F32, tag="esink")
            tri_t = P.tile([128, 128], BF16, tag="tri")
            wo_t = [P.tile([128, DIM], BF16, name=f"wo{i}", tag=f"wo{i}") for i in range(2)]
            qp = [P.tile([128, S], BF16, name=f"qp{i}", tag=f"qp{i}") for i in range(2)]
            kT2 = P.tile([128, S], BF16, tag="kT2")
            vTf = P.tile([64, S], F32, tag="vTf")
            vext = P.tile([128, NST * 64], BF16, tag="vext")
            outstk = [P.tile([128, S], BF16, name=f"os{i}", tag=f"os{i}") for i in range(2)]
            id_t = P.tile([HD, HD], F32, tag="idp")
            onesb_t = P.tile([128, 64], BF16, tag="onesb_t")
            onesf_t = P.tile([128, 64], F32, tag="onesf_t")
            bcol_t = P.tile([128, 3], F32, tag="bcol")
            cos_t = P.tile([128, S], BF16, tag="cos")
            sinr_t = P.tile([128, S], BF16, tag="sinr")
            scr = P.tile([1, 16], F32, tag="scr")
            w_t = [P.tile([128, 384], BF16, name=f"w{dc}", tag=f"w{dc}") for dc in range(NDC)]
            x_t = [P.tile([128, S], BF16, name=f"x{dc}", tag=f"x{dc}") for dc in range(NDC)]
            # weights + first x column block interleaved on the two HWDGE
            # queues so round-0 projection unblocks per-dc as data lands;
            # later column blocks follow in round order
            _xq = [nc.sync, nc.scalar]
            for dc in range(NDC):
                q = _xq[dc % 2]
                q.dma_start(w_t[dc][:], wproj[dc * 128 : (dc + 1) * 128, :])
                q.dma_start(x_t[dc][:, 0:SBLK], xT[dc * 128 : (dc + 1) * 128, 0:SBLK])
            # block 1 rides both fast queues right after block 0; blocks 2-3
            # queue strictly behind on sync so they cannot steal HBM bandwidth
            # from the critical early loads
            cs = slice(SBLK, 2 * SBLK)
            _q3 = [nc.sync, nc.scalar, nc.gpsimd, nc.sync]
            for dc in range(NDC):
                _q3[dc % 4].dma_start(x_t[dc][:, cs], xT[dc * 128 : (dc + 1) * 128, cs])
            for blk in range(2, NSB):
                cs = slice(blk * SBLK, (blk + 1) * SBLK)
                for dc in range(NDC):
                    nc.sync.dma_start(x_t[dc][:, cs], xT[dc * 128 : (dc + 1) * 128, cs])
            nc.gpsimd.dma_start(bcol_t[:], bproj[:])
            nc.gpsimd.dma_start(cos_t[:], cos2[:])
            nc.gpsimd.dma_start(sinr_t[:], sinr2[:])
            nc.gpsimd.dma_start(id_t[:], identf[:])
            nc.gpsimd.dma_start(onesb_t[:], onesb[:])
            nc.gpsimd.dma_start(onesf_t[:], onesf[:])
            nc.gpsimd.dma_start(esink_t[:], esinkc[:])
            nc.gpsimd.dma_start(tri_t[:], tri[:])
            for i in range(2):
                nc.gpsimd.dma_start(wo_t[i][:], woT[i * 128 : (i + 1) * 128, :])
            # pull the ACT Exp/Ln table load off the critical path
            nc.scalar.activation(scr[0:1, 0:3], bcol_t[0:1, 0:3], AF.Exp)
            nc.scalar.activation(scr[0:1, 0:3], scr[0:1, 0:3], AF.Ln)
            nc.scalar.activation(scr[0:1, 0:3], scr[0:1, 0:3], AF.Identity)

            # ---------------- helpers (emit instructions) ----------------

            def rope_q(i, pp, ss):
                """Evict q tile i from psum pp (one fast read), then RoPE in
                bf16 on SBUF (2x DVE mode); frees the proj psum bank early."""
                u = TMP.tile([128, SBLK], BF16, name="u", tag="u")
                nc.scalar.activation(u[:], pp, AF.Identity, bias=bcol_t[:, i : i + 1])
                t1 = TMP.tile([128, SBLK], BF16, name="t1", tag="t1")
                nc.vector.tensor_tensor(t1[:], u[:], cos_t[:, ss], op=OP.mult)
                t2 = TMP.tile([128, SBLK], BF16, name="t2", tag="t2")
                for g in range(4):
                    d0 = 32 * g
                    s0 = d0 + 32 if g % 2 == 0 else d0 - 32
                    nc.vector.tensor_tensor(
                        t2[d0 : d0 + 32, :], u[s0 : s0 + 32, :],
                        sinr_t[s0 : s0 + 32, ss], op=OP.mult,
                    )
                nc.vector.tensor_tensor(qp[i][:, ss], t1[:], t2[:], op=OP.add)

            def rope_kv(pp, ss):
                uk = TMP.tile([64, SBLK], BF16, name="uk", tag="uk")
                nc.scalar.activation(uk[:], pp[0:64, :], AF.Identity, bias=bcol_t[0:64, 2:3])
                # v rows with bias, fp32 (transposed later on PE)
                nc.scalar.activation(vTf[:, ss], pp[64:128, :], AF.Identity, bias=bcol_t[64:128, 2:3])
                tk1 = TMP.tile([64, SBLK], BF16, name="tk1", tag="tk1")
                nc.vector.tensor_tensor(tk1[:], uk[:], cos_t[0:64, ss], op=OP.mult)
                tk2 = TMP.tile([64, SBLK], BF16, name="tk2", tag="tk2")
                nc.vector.tensor_tensor(
                    tk2[0:32, :], uk[32:64, :], sinr_t[32:64, ss], op=OP.mult
                )
                nc.vector.tensor_tensor(
                    tk2[32:64, :], uk[0:32, :], sinr_t[0:32, ss], op=OP.mult
                )
                nc.vector.tensor_tensor(kT2[0:64, ss], tk1[:], tk2[:], op=OP.add)
                nc.vector.tensor_copy(kT2[64:128, ss], kT2[0:64, ss])

            def transp_round(r):
                """Transpose this round's 4 v tiles into vext via the S ring."""
                tp = PS_S.tile([128, 1024], F32, name="tp", tag="s")
                for j in range(4):
                    t = 4 * r + j
                    nc.tensor.transpose(
                        tp[:, j * 64 : (j + 1) * 64],
                        vTf[:, t * 128 : (t + 1) * 128],
                        id_t[:],
                    )
                    nc.vector.tensor_copy(
                        vext[:, t * 64 : (t + 1) * 64], tp[:, j * 64 : (j + 1) * 64]
                    )

            pso_cur = [None]
            den_cur = [None]
            ptt_store = {}

            def s_iter(b, t):
                """Scores + exp for sk-tile t of block b (both head pairs)."""
                off = 128 * (t - 4 * b) if t >= 4 * b else 0
                n0 = b * SBLK + off
                ptts = []
                for pi in range(2):
                    sbt = PS_S.tile([128, 1024], F32, name="sbt", tag="s")
                    # lane0 at [off:512], lane1 packed at [512:1024-off] so the
                    # exp input region is contiguous (no stale-data gap)
                    for lane, (c0, c1) in enumerate([(off, 512), (512, 1024 - off)]):
                        nc.tensor.matmul(
                            sbt[:, c0:c1],
                            kT2[64 * lane : 64 * lane + 64, t * 128 : (t + 1) * 128],
                            qp[pi][64 * lane : 64 * lane + 64, n0 : (b + 1) * SBLK],
                            start=True, stop=True,
                            tile_position=(64 * lane, 0),
                        )
                    ptt = PT.tile([128, 1024], BF16, name="ptt", tag="pt")
                    nc.scalar.activation(
                        ptt[:, off : 1024 - off], sbt[:, off : 1024 - off],
                        AF.Exp, scale=SCALE,
                    )
                    if t >= 4 * b:
                        for c in (off, 512):
                            nc.vector.tensor_tensor(
                                ptt[:, c : c + 128], ptt[:, c : c + 128],
                                tri_t[:], op=OP.mult,
                            )
                    ptts.append(ptt)
                ptt_store[(b, t)] = ptts

            def pv_iter(b, t):
                """PV accumulation + denominators for sk-tile t of block b."""
                off = 128 * (t - 4 * b) if t >= 4 * b else 0
                ptts = ptt_store.pop((b, t))
                if t == 0:
                    pso_cur[0] = [
                        PS_O.tile([128, SBLK], F32, name=f"pso{pi}", tag="o")
                        for pi in range(2)
                    ]
                    den_cur[0] = PS_D.tile([128, SBLK], F32, name="den", tag="d")
                pso, den = pso_cur[0], den_cur[0]
                vx = vext[:, t * 64 : (t + 1) * 64]
                first = t == 0
                last = t == 4 * b + 3
                for pi in range(2):
                    nc.tensor.matmul(
                        pso[pi][0:64, off:SBLK],
                        vx, ptts[pi][:, off:512],
                        start=first, stop=last, tile_position=(0, 0),
                    )
                    # skip_group_check: the sim's zero-region bookkeeping
                    # mis-translates partition-offset APs (aliases partitions
                    # 8..71); flags here mirror the checked even-half matmul
                    nc.tensor.matmul(
                        pso[pi][64:128, off:SBLK],
                        vx, ptts[pi][:, 512 : 1024 - off],
                        start=first, stop=last, tile_position=(0, 64),
                        skip_group_check=True,
                    )
                for h in range(QH):
                    pi, odd = h // 2, h % 2
                    # M=32 (same cycles as M=1) so the whole den bank is
                    # initialized for the full-tile renorm read later
                    nc.tensor.matmul(
                        den[32 * h : 32 * h + 32, off:SBLK],
                        onesb_t[:, 0:32],
                        ptts[pi][:, 512 * odd + off * (1 - odd) : 512 + 512 * odd - off * odd],
                        start=first, stop=last, tile_position=(0, 32 * h),
                        skip_group_check=(h > 0),
                    )

            rinv_cur = [None]

            def renorm_a(b):
                """r = sumexp + e^sink; rinv = 1/r (ACT chain, no PE)."""
                den = den_cur[0]
                rowb = RN.tile([128, SBLK], F32, name="rowb", tag="rowb")
                nc.vector.tensor_scalar_add(rowb[:], den[:], esink_t[:])
                lnr = RN.tile([128, SBLK], F32, name="lnr", tag="lnr")
                nc.scalar.activation(lnr[:], rowb[:], AF.Ln)
                rinv = RN.tile([128, SBLK], F32, name="rinv", tag="rinv")
                nc.scalar.activation(rinv[:], lnr[:], AF.Exp, scale=-1.0)
                rinv_cur[0] = rinv

            def renorm_b(b):
                """outstk = pso * rinv broadcast (K=1 matmuls + DVE mults)."""
                bs = slice(b * SBLK, (b + 1) * SBLK)
                pso, rinv = pso_cur[0], rinv_cur[0]
                rbt = PS_S.tile([128, 1024], F32, name="rbt", tag="s")
                for h in range(QH):
                    pi, odd = h // 2, h % 2
                    nc.tensor.matmul(
                        rbt[64 * odd : 64 * odd + 64, 512 * pi : 512 * pi + 512],
                        onesf_t[32 * h : 32 * h + 1, :], rinv[32 * h : 32 * h + 1, :],
                        start=True, stop=True, tile_position=(32 * h, 64 * odd),
                    )
                rbs = RN.tile([128, 1024], F32, name="rbs", tag="rbs")
                nc.vector.tensor_copy(rbs[:, 0:512], rbt[:, 0:512])
                nc.scalar.copy(rbs[:, 512:1024], rbt[:, 512:1024])
                for pi in range(2):
                    nc.vector.tensor_tensor(
                        outstk[pi][:, bs], pso[pi][:],
                        rbs[:, 512 * pi : 512 * pi + 512], op=OP.mult,
                    )

            def renorm_tail(b):
                renorm_a(b)
                renorm_b(b)

            def outproj_tile(st, dpair, split_cast=False, pool=None):
                """Project sq tile st for output column pair dpair (2x512)."""
                psf = None
                if pool is None:
                    psf = PS_S.tile([128, 1024], F32, name="psf", tag="s")
                for half in range(2):
                    db = 2 * dpair + half
                    ds = slice(db * SBLK, (db + 1) * SBLK)
                    if pool is not None:
                        psf_h = pool.tile([128, SBLK], F32, name="psfh", tag="f")
                        hs = psf_h[:]
                    else:
                        hs = psf[:, 512 * half : 512 * half + 512]
                    nc.tensor.matmul(
                        hs,
                        outstk[0][:, st * 128 : (st + 1) * 128], wo_t[0][:, ds],
                        start=True, stop=False,
                    )
                    nc.tensor.matmul(
                        hs,
                        outstk[1][:, st * 128 : (st + 1) * 128], wo_t[1][:, ds],
                        start=False, stop=True,
                    )
                    ot = EV.tile([128, SBLK], BF16, name="ot", tag="ev")
                    if split_cast and half == 1:
                        nc.scalar.copy(ot[:], hs)
                    else:
                        nc.vector.tensor_copy(ot[:], hs)
                    _oq = nc.sync if half == 0 else nc.gpsimd
                    _oq.dma_start(out[st * 128 : (st + 1) * 128, ds], ot[:])

            def proj_group(g, ss, pp):
                c0 = 128 * g if g < 2 else 256
                c1 = c0 + 128
                for dc in range(NDC):
                    nc.tensor.matmul(
                        pp, w_t[dc][:, c0:c1], x_t[dc][:, ss],
                        start=(dc == 0), stop=(dc == NDC - 1),
                    )

            # ---------------- round 0: projection of block 0 ----------------
            ss0 = slice(0, SBLK)
            s_q = PS_S.tile([128, 1024], F32, name="s_q", tag="s")
            s_k = PS_S.tile([128, 1024], F32, name="s_k", tag="s")
            for dc in range(NDC):
                nc.tensor.matmul(
                    s_q[:, 0:512], w_t[dc][:, 0:128], x_t[dc][:, ss0],
                    start=(dc == 0), stop=(dc == NDC - 1),
                )
                nc.tensor.matmul(
                    s_q[:, 512:1024], w_t[dc][:, 128:256], x_t[dc][:, ss0],
                    start=(dc == 0), stop=(dc == NDC - 1),
                )
                nc.tensor.matmul(
                    s_k[:, 0:512], w_t[dc][:, 256:384], x_t[dc][:, ss0],
                    start=(dc == 0), stop=(dc == NDC - 1),
                )
            rope_q(0, s_q[:, 0:512], ss0)
            rope_q(1, s_q[:, 512:1024], ss0)
            rope_kv(s_k[:, 0:512], ss0)
            transp_round(0)
            s_iter(0, 0)
            s_iter(0, 1)

            # ------- rounds 1..3: proj r + attn r-1, score-lead pipeline -----
            for r in range(1, NSB):
                b = r - 1
                nt = 4 * b + 4
                ssr = slice(r * SBLK, (r + 1) * SBLK)

                fillers = []

                def mk_proj(g, ssr=ssr):
                    def f():
                        pp = PS_P.tile([128, SBLK], F32, name="pp", tag="p")
                        proj_group(g, ssr, pp[:])
                        if g < 2:
                            rope_q(g, pp[:], ssr)
                        else:
                            rope_kv(pp[:], ssr)
                    return f

                for g in range(3):
                    fillers.append(mk_proj(g))
                if b >= 1:
                    for st in range(4 * (b - 1), 4 * (b - 1) + 4):
                        for dp in range(2):
                            fillers.append(
                                lambda st=st, dp=dp: outproj_tile(st, dp)
                            )
                fi = 0
                for i in range(nt):
                    if i + 2 < nt:
                        s_iter(b, i + 2)
                    pv_iter(b, i)
                    while fi < len(fillers) and (fi + 1) * nt <= (i + 1) * len(fillers):
                        fillers[fi]()
                        fi += 1
                while fi < len(fillers):
                    fillers[fi]()
                    fi += 1
                renorm_tail(b)
                transp_round(r)
                s_iter(r, 0)
                s_iter(r, 1)

            # ------- block 3: lead-4 score prefetch so the last PV iters
            # run dense (exps pre-done), outproj(2) tiles as spacers -------
            ofill = [
                (lambda st=st, dp=dp: outproj_tile(st, dp))
                for st in range(8, 12) for dp in range(2)
            ]
            ofill[0]()
            s_iter(3, 2)
            ofill[1]()
            s_iter(3, 3)
            for i in range(16):
                pv_iter(3, i)
                if i + 4 < 16:
                    s_iter(3, i + 4)
                if i in (1, 3, 5, 7, 9, 11):
                    ofill[2 + i // 2]()

            # ---------------- final renorm + output projection ----------------
            # scratch matmuls keep the PE (and HAM clock) busy while the
            # serial renorm chain runs on ACT/DVE; output is never read
            scratch = PS_S.tile([128, 1024], F32, name="scratch", tag="s")
            for w in range(16):
                nc.tensor.matmul(
                    scratch[0:64, 0:512], onesb_t[:, 0:64], qp[0][0:128, 0:512],
                    start=True, stop=True,
                )
            renorm_tail(3)
            PS_D.release()
            PS_O.release()
            PS_P.release()
            PS_F = tc.alloc_tile_pool(name="ps_f", bufs=4, space="PSUM")
            for st in range(12, 16):
                for dp in range(2):
                    outproj_tile(st, dp, split_cast=True, pool=PS_F)
            PS_F.release()

    _fix_range_clears(nc)
    if split_waits:
        _split_excess_waits(nc)
    return nc


_nc_cache = [None]


def kernel(**inputs):
    in_maps = prep_inputs(inputs)
    if _nc_cache[0] is None:
        _nc_cache[0] = build_nc()
    nc = _nc_cache[0]
    res = run_bass_kernel_spmd(nc, in_maps, list(range(NCORES)))
    acc = res.results[0]["out"].astype(np.float32)
    for i in range(1, NCORES):
        acc = acc + res.results[i]["out"].astype(np.float32)
    acc = acc + np.asarray(inputs["wo_b"], np.float32).reshape(1, DIM)
    return acc.reshape(B, S, DIM)
